# revision 13
# baseline (speedup 1.0000x reference)
"""AGCRN cell with per-node MLP-generated gate weights, on 8 TRN2 NeuronCores.

Math (reference):
    combined = adj @ concat([x, h], -1)          # [N, 257]
    cg = combined[nodes_ind]                     # [M, 257]
    gate(f, q, W, b) = einsum('ni,nd,dio->no', f, q, W) + q @ b
    r = sigmoid(gate(cg, q, W_r, b_r)); u = sigmoid(gate(cg, q, W_u, b_u))
    cn = [x_sel, r * h_sel]                      # [M, 257]
    cand = tanh(gate(cn, q, W_c, b_c))
    new_h = (1 - u) * (r * h_sel) + u * cand     # [M, 128]

M rows shard across 8 cores (Mc = 256 rows/core), W_* replicated, no
collectives. Transposed orientation on device: [feat, n].

fp8 strategy (rel err ~0.0054 vs 2e-2 budget, validated in numpy):
  - adj matmul in fp8 e4m3 DoubleRow (A pre-scaled by N so entries ~U(0,1);
    C = concat(x,h) is ~randn): cg feeds only the r/u gates, whose preacts
    are dominated by the bias term, so 2.5% cg noise is harmless.
  - r/u gate matmuls in fp8 DoubleRow: z_ru scaled by S=128 (fp8-friendly
    rms ~1.1), biases/tails scaled by S in bf16, sigmoid applies 1/S.
  - gate c stays bf16 end-to-end (tanh preact std ~5.3 would amplify fp8
    noise ~4x past the error budget).
  DoubleRow measured at 109ns per k-tile PAIR (= 2.0x bf16, LDWEIGHTS
  hidden); fp8 also halves the C/AT/Wr/Wu DMA bytes (10MB -> 6.4MB).

Per-core schedule (DMA ~6.4MB = 18us, PE ~19us, DVE ~24us):
  - q_bcast [128, 8192] built by k=1 ones-matmuls on the PE (16 x 512 cols
    -> psum) + psum->SBUF copies split DVE/ACT.  No GpSimd at all (its
    ucode load + 12us broadcast was the baseline's critical-path stall).
    The broadcasts interleave with the DMA-paced adj DoubleRow pairs so
    the HAM half-clock warm-up window does useful work.
  - z chunks are bf16 on DVE (fp8 DVE output measured 2x slower), the
    r/u chunks are converted bf16->fp8 on the otherwise idle ACT engine.
  - All W streams are chunked in exact consumption order: Wr/Wu blocks
    (k-chunk, ihalf)-major matching the (k, ih) gate loop, then Wc.
  - Tail: sigmoid r -> rh -> zc1 chunks on DVE interleaved with the
    gate-c h-part matmuls; (1-u)rh precomputed under them.
"""

import os
import sys

sys.path.insert(0, "/opt/trn_rl_repo")

import numpy as np
import ml_dtypes
from ml_dtypes import bfloat16

import concourse.bass as bass
import concourse.tile as tile
from concourse import bacc, mybir
from concourse.bass_utils import run_bass_kernel_spmd

NC = 8
N = 4096
M = 2048
Mc = M // NC  # 256 rows per core
QD = 32
O = 128
S = 128.0  # fp8 scale folded into q_bcast / biases; activations apply 1/S
KROWS = QD * 256  # 8192 z rows (d, i<256)
ADJ_CH = [4, 4, 8, 8, 8]  # adj k-tile chunks (32 total), small first
BF16 = mybir.dt.bfloat16
F8 = mybir.dt.float8e4
F32 = mybir.dt.float32
AF = mybir.ActivationFunctionType
DR = mybir.MatmulPerfMode.DoubleRow
SMW = 3 * 256 + 6 * 128  # packed [32, x] smalls: qT, b_r/u/c, xtail, cgL, Wt_r/u/c

_COMPILED = None


def _dr(ap, half):
    """2D AP slice -> 3D DoubleRow AP [128, 2, half]."""
    return ap.rearrange("p (two m) -> p two m", two=2)


def _build():
    nc = bacc.Bacc("TRN2", target_bir_lowering=False, debug=False, num_devices=NC)
    d_AT = nc.dram_tensor("AT", [128, 32 * Mc], F8, kind="ExternalInput").ap()
    d_C0 = nc.dram_tensor("C0", [128, 32 * 128], F8, kind="ExternalInput").ap()
    d_C1 = nc.dram_tensor("C1", [128, 32 * 128], F8, kind="ExternalInput").ap()
    d_Wr = nc.dram_tensor("Wr", [128, 64 * O], F8, kind="ExternalInput").ap()
    d_Wu = nc.dram_tensor("Wu", [128, 64 * O], F8, kind="ExternalInput").ap()
    d_Wc = nc.dram_tensor("Wc", [128, 64 * O], BF16, kind="ExternalInput").ap()
    d_SM = nc.dram_tensor("SM", [QD, SMW], BF16, kind="ExternalInput").ap()
    d_qTf = nc.dram_tensor("qTflat", [1, KROWS], BF16, kind="ExternalInput").ap()
    d_xT = nc.dram_tensor("xT", [128, Mc], BF16, kind="ExternalInput").ap()
    d_hT = nc.dram_tensor("hT", [128, Mc], F32, kind="ExternalInput").ap()
    d_out = nc.dram_tensor("out", [O, Mc], F32, kind="ExternalOutput").ap()
    DBG = bool(os.environ.get("BASS_KERNEL_DEBUG"))
    if DBG:
        d_dbg_qbc = nc.dram_tensor("dbg_qbc", [128, KROWS], BF16, kind="ExternalOutput").ap()
        d_dbg_cgT = nc.dram_tensor("dbg_cgT", [128, 2 * Mc], BF16, kind="ExternalOutput").ap()
        d_dbg_r = nc.dram_tensor("dbg_r", [128, Mc], F32, kind="ExternalOutput").ap()
        d_dbg_u = nc.dram_tensor("dbg_u", [128, Mc], F32, kind="ExternalOutput").ap()
        d_dbg_zg = nc.dram_tensor("dbg_zg", [128, 2048], F8, kind="ExternalOutput").ap()
        d_dbg_cand = nc.dram_tensor("dbg_cand", [128, Mc], F32, kind="ExternalOutput").ap()

    with tile.TileContext(nc) as tc:
        with (
            tc.tile_pool(name="res", bufs=1) as res,
            tc.tile_pool(name="psum", bufs=1, space=bass.MemorySpace.PSUM) as pp,
        ):
            # --- ACT table preload (sigmoid_and_others: copy/sigmoid/tanh)
            warm = res.tile([1, 8], F32, name="warm")
            nc.vector.memset(warm[:], 0.0)
            warm2 = res.tile([1, 8], F32, name="warm2")
            nc.scalar.activation(warm2[:], warm[:], AF.Sigmoid)
            ones = res.tile([1, 128], BF16, name="ones")
            nc.vector.memset(ones[:], 1.0)

            # --- DMA: sync queue = smalls + C + Wr + Wc0 + out
            #          scalar queue = AT + Wu + hT + Wc1
            qTf_sb = res.tile([1, KROWS], BF16, name="qTf_sb")
            nc.sync.dma_start(qTf_sb[:], d_qTf[:])
            sm_sb = res.tile([QD, SMW], BF16, name="sm_sb")
            nc.sync.dma_start(sm_sb[:], d_SM[:])
            xT_sb = res.tile([128, Mc], BF16, name="xT_sb")
            nc.sync.dma_start(xT_sb[:], d_xT[:])

            C0_sb, C1_sb, AT_sb = [], [], []
            off = 0
            for j, w in enumerate(ADJ_CH):
                c0 = res.tile([128, w * 128], F8, name=f"C0_sb{j}")
                nc.sync.dma_start(c0[:], d_C0[:, off * 128 : (off + w) * 128])
                C0_sb.append(c0)
                c1 = res.tile([128, w * 128], F8, name=f"C1_sb{j}")
                nc.sync.dma_start(c1[:], d_C1[:, off * 128 : (off + w) * 128])
                C1_sb.append(c1)
                at = res.tile([128, w * Mc], F8, name=f"AT_sb{j}")
                nc.scalar.dma_start(at[:], d_AT[:, off * Mc : (off + w) * Mc])
                AT_sb.append(at)
                off += w

            # W r/u: one SBUF tile each, streamed in 8 chunks of [128, 1024]
            # (chunk index = k*2 + ih, matching the gate loop order)
            Wr_sb = res.tile([128, 64 * O], F8, name="Wr_sb")
            Wu_sb = res.tile([128, 64 * O], F8, name="Wu_sb")
            for ch in range(8):
                sl = slice(ch * 8 * O, (ch + 1) * 8 * O)
                nc.sync.dma_start(Wr_sb[:, sl], d_Wr[:, sl])
                nc.scalar.dma_start(Wu_sb[:, sl], d_Wu[:, sl])
            # W c: ih0 block then ih1 block, 4 chunks each
            Wc_sb = res.tile([128, 64 * O], BF16, name="Wc_sb")
            for ch in range(4):
                sl = slice(ch * 8 * O, (ch + 1) * 8 * O)
                nc.sync.dma_start(Wc_sb[:, sl], d_Wc[:, sl])
            hT_sb = res.tile([128, Mc], F32, name="hT_sb")
            nc.scalar.dma_start(hT_sb[:], d_hT[:])
            for ch in range(4, 8):
                sl = slice(ch * 8 * O, (ch + 1) * 8 * O)
                nc.scalar.dma_start(Wc_sb[:, sl], d_Wc[:, sl])

            qT_sb = sm_sb[:, 0:Mc]
            b_sb = {g: sm_sb[:, Mc + i * O : Mc + (i + 1) * O] for i, g in enumerate("ruc")}
            xtail_sb = sm_sb[:, Mc + 3 * O : 2 * Mc + 3 * O]
            cgL_sb = sm_sb[:, 2 * Mc + 3 * O : 3 * Mc + 3 * O]
            Wt_sb = {
                g: sm_sb[:, 3 * Mc + (3 + i) * O : 3 * Mc + (4 + i) * O]
                for i, g in enumerate("ruc")
            }

            # --- PSUM tiles. One bank per accumulation group: matmul
            # start=True resets the whole bank, so groups must not share.
            qp = [pp.tile([128, 512], F32, name=f"qp{i}") for i in range(3)]
            pr = pp.tile([128, Mc], F32, name="pr")[:]
            pu = pp.tile([128, Mc], F32, name="pu")[:]
            pc = pp.tile([128, Mc], F32, name="pc")[:]
            pcg = [pp.tile([128, Mc], F32, name=f"pcg{i}")[:] for i in range(2)]

            q_bc = res.tile([128, KROWS], BF16, name="q_bc")

            def qbc_mm(i):
                # 512-col k=1 broadcast of qTf chunk i into qp[i%3]
                nc.tensor.matmul(
                    qp[i % 3][:],
                    ones[:],
                    qTf_sb[:, i * 512 : (i + 1) * 512],
                    start=True,
                    stop=True,
                )

            def qbc_copy(i):
                # psum -> SBUF; chunks 0/2/4/6 on DVE (early), rest on ACT
                if i in (0, 2, 4, 6):
                    nc.vector.tensor_copy(q_bc[:, i * 512 : (i + 1) * 512], qp[i % 3][:])
                else:
                    nc.scalar.activation(
                        q_bc[:, i * 512 : (i + 1) * 512], qp[i % 3][:], AF.Copy
                    )

            def adj_chunk(j):
                # DoubleRow pairs over this chunk's k-tiles, both m-groups
                w = ADJ_CH[j]
                base = sum(ADJ_CH[:j])
                for p in range(w // 2):
                    t = base // 2 + p
                    atap = _dr(AT_sb[j][:, p * 2 * Mc : (p + 1) * 2 * Mc], Mc)
                    for gi, C_sb in ((0, C0_sb), (1, C1_sb)):
                        nc.tensor.matmul(
                            pcg[gi],
                            _dr(C_sb[j][:, p * 256 : (p + 1) * 256], 128),
                            atap,
                            start=(t == 0),
                            stop=(t == 15),
                            perf_mode=DR,
                        )

            # --- PE phase 1: q_bc broadcasts interleaved with bias + adj.
            # Each copy is emitted right after its matmul (the 3 qp tiles
            # rotate; emitting copies late would read clobbered psum).
            # Early chunks ride the half-clock warm-up window between
            # DMA-paced adj chunks.
            for i in (0, 1, 2, 3):
                qbc_mm(i)
                qbc_copy(i)
            nc.tensor.matmul(pr, b_sb["r"], qT_sb, start=True, stop=False)
            nc.tensor.matmul(pu, b_sb["u"], qT_sb, start=True, stop=False)
            nc.tensor.matmul(pc, b_sb["c"], qT_sb, start=True, stop=False)
            adj_chunk(0)
            for i in (4, 5):
                qbc_mm(i)
                qbc_copy(i)
            adj_chunk(1)
            for i in (6, 7):
                qbc_mm(i)
                qbc_copy(i)
            adj_chunk(2)
            for i in (8, 9):
                qbc_mm(i)
                qbc_copy(i)
            adj_chunk(3)
            for i in (10, 11):
                qbc_mm(i)
                qbc_copy(i)
            adj_chunk(4)
            for i in (12, 13, 14, 15):
                qbc_mm(i)
                qbc_copy(i)

            # --- engine-queue-ordered non-PE work.
            # DVE: q0/q2 copies + zgt/zct + zc0[0,1] early, cgT, zg stream,
            #      zc0[2,3], rh, zc1, final combine.
            # ACT: q1/q3/q4..q7 copies, zg fp8 converts, sigmoids, tanh.
            def zchunk(dst_ap, src_ap, k):
                # dst[p, dd*Mc + n] = src[p, n] * q_bc[p, k*2048 + dd*Mc + n]
                nc.vector.tensor_mul(
                    dst_ap.rearrange("p (a b) -> p a b", b=Mc),
                    src_ap.unsqueeze(1).broadcast_to((128, 8, Mc)),
                    q_bc[:, k * 2048 : (k + 1) * 2048].rearrange(
                        "p (a b) -> p a b", b=Mc
                    ),
                )

            zgt = res.tile([QD, Mc], BF16, name="zgt")
            nc.vector.tensor_mul(zgt[:], qT_sb, cgL_sb)
            zct = res.tile([QD, Mc], BF16, name="zct")
            nc.vector.tensor_mul(zct[:], qT_sb, xtail_sb)
            zc0 = [res.tile([128, 2048], BF16, name=f"zc0_{k}") for k in range(4)]
            zchunk(zc0[0][:], xT_sb[:], 0)
            zchunk(zc0[1][:], xT_sb[:], 1)

            cgT = [res.tile([128, Mc], BF16, name=f"cgT{i}") for i in range(2)]
            for i in range(2):
                nc.vector.tensor_copy(cgT[i][:], pcg[i])

            # z_ru production: bf16 on DVE -> fp8 via ACT, (k, ih) order
            zgb = [res.tile([128, 2048], BF16, name=f"zgb{i}") for i in range(3)]
            zg8 = [
                [res.tile([128, 2048], F8, name=f"zg8_{ih}_{k}") for k in range(4)]
                for ih in range(2)
            ]
            nstage = 0
            for k in range(4):
                for ih in range(2):
                    stg = zgb[nstage % 3]
                    nstage += 1
                    zchunk(stg[:], cgT[ih][:], k)
                    nc.scalar.activation(zg8[ih][k][:], stg[:], AF.Copy)
            zchunk(zc0[2][:], xT_sb[:], 2)
            zchunk(zc0[3][:], xT_sb[:], 3)

            # --- PE phase 2: r/u gates, DoubleRow, (k, ih, pair) order
            for k in range(4):
                for ih in range(2):
                    base = (k * 2 + ih) * 1024
                    for e in range(4):
                        zap = _dr(zg8[ih][k][:, e * 512 : (e + 1) * 512], Mc)
                        nc.tensor.matmul(
                            pr,
                            _dr(Wr_sb[:, base + e * 256 : base + (e + 1) * 256], O),
                            zap,
                            start=False,
                            stop=False,
                            perf_mode=DR,
                        )
                        nc.tensor.matmul(
                            pu,
                            _dr(Wu_sb[:, base + e * 256 : base + (e + 1) * 256], O),
                            zap,
                            start=False,
                            stop=False,
                            perf_mode=DR,
                        )
            nc.tensor.matmul(pr, Wt_sb["r"], zgt[:], start=False, stop=True)
            nc.tensor.matmul(pu, Wt_sb["u"], zgt[:], start=False, stop=True)

            r_sb = res.tile([128, Mc], F32, name="r_sb")
            nc.scalar.activation(r_sb[:], pr, AF.Sigmoid, scale=1.0 / S)
            u_sb = res.tile([128, Mc], F32, name="u_sb")
            nc.scalar.activation(u_sb[:], pu, AF.Sigmoid, scale=1.0 / S)

            # --- PE: gate-c x-part (bf16) while sigmoid/rh/zc1 run
            for k in range(4):
                for dd in range(8):
                    d = k * 8 + dd
                    nc.tensor.matmul(
                        pc,
                        Wc_sb[:, d * O : (d + 1) * O],
                        zc0[k][:, dd * Mc : (dd + 1) * Mc],
                        start=False,
                        stop=False,
                    )
            nc.tensor.matmul(pc, Wt_sb["c"], zct[:], start=False, stop=False)

            # --- tail: rh, zc1 chunks interleaved with gate-c h-part
            rh_f = res.tile([128, Mc], F32, name="rh_f")
            nc.vector.tensor_mul(rh_f[:], r_sb[:], hT_sb[:])
            rh_b = res.tile([128, Mc], BF16, name="rh_b")
            nc.vector.tensor_scalar_mul(rh_b[:], rh_f[:], float(N))
            zc1 = [res.tile([128, 2048], BF16, name=f"zc1_{k}") for k in range(4)]
            for k in range(4):
                zchunk(zc1[k][:], rh_b[:], k)
                for dd in range(8):
                    d = k * 8 + dd
                    nc.tensor.matmul(
                        pc,
                        Wc_sb[:, (32 + d) * O : (33 + d) * O],
                        zc1[k][:, dd * Mc : (dd + 1) * Mc],
                        start=False,
                        stop=(d == 31),
                    )
            # (1-u)*rh precomputed under the gate-c matmuls
            t1 = res.tile([128, Mc], F32, name="t1")
            nc.vector.tensor_mul(t1[:], u_sb[:], rh_f[:])
            rh1u = res.tile([128, Mc], F32, name="rh1u")
            nc.vector.tensor_sub(rh1u[:], rh_f[:], t1[:])
            cand_sb = res.tile([128, Mc], F32, name="cand_sb")
            nc.scalar.activation(cand_sb[:], pc, AF.Tanh, scale=1.0 / S)

            t2 = res.tile([128, Mc], F32, name="t2")
            nc.vector.tensor_mul(t2[:], u_sb[:], cand_sb[:])
            outT = res.tile([128, Mc], F32, name="outT")
            nc.vector.tensor_add(outT[:], rh1u[:], t2[:])
            nc.sync.dma_start(d_out[:], outT[:])
            if DBG:
                nc.sync.dma_start(d_dbg_qbc[:], q_bc[:])
                nc.sync.dma_start(d_dbg_cgT[:, 0:Mc], cgT[0][:])
                nc.sync.dma_start(d_dbg_cgT[:, Mc : 2 * Mc], cgT[1][:])
                nc.sync.dma_start(d_dbg_r[:], r_sb[:])
                nc.sync.dma_start(d_dbg_u[:], u_sb[:])
                nc.sync.dma_start(d_dbg_zg[:], zg8[0][0][:])
                nc.sync.dma_start(d_dbg_cand[:], cand_sb[:])

    nc.compile()
    return nc


def _get_compiled():
    global _COMPILED
    if _COMPILED is None:
        _COMPILED = _build()
    return _COMPILED


def _pmajor(a, width):
    """[n_tiles*128, width] row-major -> [128, n_tiles*width] partition-major."""
    nt = a.shape[0] // 128
    return np.ascontiguousarray(
        a.reshape(nt, 128, width).transpose(1, 0, 2).reshape(128, nt * width)
    )


def _prep_inputs(x, h, query_vectors, adj, nodes_ind, W_u, b_u, W_r, b_r, W_c, b_c):
    idx = np.asarray(nodes_ind).astype(np.int64)
    f32 = np.float32
    f8 = ml_dtypes.float8_e4m3
    x = np.asarray(x, f32)
    h = np.asarray(h, f32)
    q = np.asarray(query_vectors, f32)
    adj = np.asarray(adj, f32)

    C_dev = np.concatenate([x[:, :128], h], axis=1)  # [N, 256]
    C0_pm = _pmajor(C_dev[:, :128], 128).astype(f8)  # [128, 32*128]
    C1_pm = _pmajor(C_dev[:, 128:], 128).astype(f8)
    A_sel = adj[idx]  # [M, N]
    x_sel = x[idx]
    h_sel = h[idx]
    cg_L = A_sel @ x[:, 128]  # [M] leftover mixed feature, on host

    def flat_main(W):
        W = np.asarray(W, f32)
        main = np.concatenate([W[:, :128, :], W[:, 129:, :]], axis=1).reshape(
            QD * 256, O
        )
        return main.reshape(QD * 2, 128, O)  # tile index = d*2 + ihalf

    def order_ru(tiles):
        # (k-chunk, ih, pair, half) order for the (k, ih, e) DR gate loop
        order = []
        for k in range(4):
            for ih in range(2):
                for e in range(4):
                    for jj in range(2):
                        d = 8 * k + 2 * e + jj
                        order.append(d * 2 + ih)
        return tiles[order].reshape(QD * 256, O)

    def order_c(tiles):
        # ihalf-major: all (d, 0) tiles then all (d, 1)
        return np.concatenate([tiles[0::2], tiles[1::2]], axis=0).reshape(
            QD * 256, O
        )

    Wr_pm = _pmajor(order_ru(flat_main(W_r)), O).astype(f8)
    Wu_pm = _pmajor(order_ru(flat_main(W_u)), O).astype(f8)
    Wc_pm = _pmajor(order_c(flat_main(W_c)), O).astype(bfloat16)
    Wt = {g: np.ascontiguousarray(np.asarray(W, f32)[:, 128, :]).astype(bfloat16)
          for g, W in (("r", W_r), ("u", W_u), ("c", W_c))}
    bf_ = {g: (np.asarray(b, f32) * S).astype(bfloat16)
           for g, b in (("r", b_r), ("u", b_u), ("c", b_c))}

    in_maps = []
    for c in range(NC):
        sl = slice(c * Mc, (c + 1) * Mc)
        qT = np.ascontiguousarray(q[sl].T).astype(bfloat16)  # [32, 256]
        AT = np.ascontiguousarray(A_sel[sl].T * np.float32(N))  # [4096, 256]
        SM = np.concatenate(
            [
                qT,
                bf_["r"],
                bf_["u"],
                bf_["c"],
                np.broadcast_to(x_sel[sl, 128] * S, (QD, Mc)).astype(bfloat16),
                np.broadcast_to(cg_L[sl] * S, (QD, Mc)).astype(bfloat16),
                Wt["r"],
                Wt["u"],
                Wt["c"],
            ],
            axis=1,
        )
        in_maps.append(
            {
                "AT": _pmajor(AT, Mc).astype(f8),
                "C0": C0_pm,
                "C1": C1_pm,
                "Wr": Wr_pm,
                "Wu": Wu_pm,
                "Wc": Wc_pm,
                "SM": np.ascontiguousarray(SM),
                "qTflat": (qT.reshape(1, KROWS).astype(f32) * (S / N)).astype(bfloat16),
                "xT": np.ascontiguousarray(x_sel[sl, :128].T * np.float32(N)).astype(bfloat16),
                "hT": np.ascontiguousarray(h_sel[sl].T).astype(f32),
            }
        )
    return in_maps


def run(inputs: dict, trace: bool = False):
    nc = _get_compiled()
    in_maps = _prep_inputs(**inputs)
    res = run_bass_kernel_spmd(nc, in_maps, core_ids=list(range(NC)), trace=trace)
    shards = [res.results[c]["out"].T for c in range(NC)]  # each [256, 128]
    out = np.concatenate(shards, axis=0).astype(np.float32)  # [M, 128]
    return out, res


def kernel(**inputs) -> np.ndarray:
    out, _ = run(inputs, trace=bool(os.environ.get("BASS_KERNEL_TRACE")))
    return out
